# revision 26
# baseline (speedup 1.0000x reference)
"""BioSSMMixer distributed Trainium2 kernel (8 NeuronCores), v2.

Sharding: channel dimension D is split across the 8 cores (the SSM scan is
diagonal in D, so each core scans its own 128 channels with no cross-core
state). The final W_out projection contracts the full D, so the gate tensor
g = y_sp * silu(z) is exchanged with an AllToAll (d-shards -> t-shards) and
each core computes the output rows for its own T/8 slice.

v2 engine plan (v1 ran nearly everything heavy on DVE, which saturated at
~107% while GpSimd/ACT idled):
- DVE keeps the 16 recurrence scans per batch row (tensor_tensor_scan only
  lowers on DVE; 2 cycles/elem is its hard floor) plus the Bm input muls.
- GpSimd runs the s*Cm products and half the y accumulation chains.
- The Activation engine converts PSUM f32 broadcast results to bf16 SBUF
  (Copy lives in every act table, so no table swap next to the Exp decays);
  with bf16-only operands DVE tensor_tensor ops hit the 2x_1p perf mode.
- The -h residual is folded into the output matmul as one extra accumulating
  matmul (identity stationary, negated-h moving), removing the tail
  subtractions and the f32 hres load.
- b=0's output stage is emitted mid-b=1-n-loop so PE never stalls on the
  AllToAll; DMA issue is spread over the three DGE-capable queues with b=0's
  h tiles first.

Host-side prep (not part of HW exec time): W_xd = W_xz[:, :D] @ W_dt is
folded so dt comes directly from h; h is pre-transposed to [D, B*T] bf16.
"""

import os
import numpy as np
import ml_dtypes

B, T, D, N = 2, 1024, 1024, 16
NCORES = 8
DL = D // NCORES        # 128 channels per core
TL = T // NCORES        # 128 timesteps per core (output slice)
R = B * T               # 2048 rows, b-major: row = b*T + t
KT = D // 128           # 8 contraction tiles
H = 512                 # psum half-tile

BF16 = ml_dtypes.bfloat16

# Tuning knobs (engine placement).
Z_POS = 11              # first n at which a z-projection half is emitted
PROJ1_POS = 4           # n (in b=0 loop) where b=1's projections are emitted
# trailing n's whose s*Cm + accumulate run on DVE instead of GpSimd: GpSimd
# is ~3.3x slower per op than 2x-mode DVE, so its 2-ops-per-n chain falls
# behind the scan cadence and stalls everyone via the s/tmp pool slots
DVE_TAIL = (2, 5)
PB_BUFS = 4             # PSUM ring slots shared by all projection/bcast tiles

# Filled by kernel() when KERNEL_TRACE=1: exec_time_ns, trace tmpdir.
LAST = {}

_GRAPH_CACHE = {}


def _patch_act_tables():
    """Order activation tables so Exp and Ln resolve to the combined
    natural_log_exp_and_others table (otherwise the table-load pass
    ping-pongs between exp_and_others and natural_log)."""
    import concourse.hw_specs as hw_specs
    import concourse.bacc as bacc_mod
    orig = hw_specs.get_activation_tables.__wrapped__
    import functools

    @functools.cache
    def reordered(arch):
        import concourse.mybir as mybir
        Act = mybir.ActivationFunctionType
        t = {k: set(v) for k, v in orig(arch).items()}
        if "natural_log_exp_and_others" in t:
            for k in ("exp_and_others", "exp_and_friends"):
                t.get(k, set()).discard(Act.Exp)
            t.get("natural_log", set()).discard(Act.Ln)
        return t

    hw_specs.get_activation_tables = reordered
    bacc_mod.get_activation_tables = reordered


def _build_graph():
    if "nc" in _GRAPH_CACHE:
        return _GRAPH_CACHE["nc"]

    import concourse.bacc as bacc
    import concourse.mybir as mybir
    from concourse import tile

    if os.environ.get('ACT_PATCH', '1') == '1':
        _patch_act_tables()

    f32 = mybir.dt.float32
    bf16 = mybir.dt.bfloat16
    Alu = mybir.AluOpType
    Act = mybir.ActivationFunctionType

    nc = bacc.Bacc(
        "TRN2",
        target_bir_lowering=False,
        debug=False,
        enable_asserts=True,
        num_devices=NCORES,
    )

    hT_d = nc.dram_tensor("hT", [B, KT, 128, T], bf16, kind="ExternalInput")
    wpack_d = nc.dram_tensor("wpack", [D, 3 * DL + 2 * N], bf16,
                             kind="ExternalInput")
    wout_d = nc.dram_tensor("wout", [D, D], bf16, kind="ExternalInput")
    acol_d = nc.dram_tensor("acol", [DL, N], f32, kind="ExternalInput")
    bdt_d = nc.dram_tensor("bdt", [DL, 1], f32, kind="ExternalInput")
    dsk_d = nc.dram_tensor("dsk", [DL, 1], f32, kind="ExternalInput")
    nvth_d = nc.dram_tensor("nvth", [DL, 1], f32, kind="ExternalInput")
    nhres_d = nc.dram_tensor("nhres", [B, TL, D], bf16, kind="ExternalInput")
    sel_d = nc.dram_tensor("sel", [2 * N, 2 * N * 128], bf16,
                           kind="ExternalInput")
    out_d = nc.dram_tensor("out", [B, TL, D], f32, kind="ExternalOutput")

    WP = 3 * DL + 2 * N

    with tile.TileContext(nc) as tc:
        with (
            tc.tile_pool(name="const", bufs=1) as cpool,
            tc.tile_pool(name="work", bufs=1) as wpool,
            tc.tile_pool(name="sc", bufs=4) as scpool,
            tc.tile_pool(name="px", bufs=2, space="PSUM") as pxpool,
            tc.tile_pool(name="dram", bufs=1, space="DRAM") as dpool,
        ):
            # ---- constant + input loads (b0's h first, spread queues) ----
            hT = cpool.tile([128, KT, R], bf16)
            wpk = cpool.tile([128, KT, WP], bf16)
            acol = cpool.tile([DL, N], f32)
            bdt = cpool.tile([DL, 1], f32)
            dsk = cpool.tile([DL, 1], f32)
            nvth = cpool.tile([DL, 1], f32)
            sel = cpool.tile([2 * N, 2 * N * 128], bf16)
            nhres = cpool.tile([TL, B, D], bf16)
            wout = cpool.tile([128, KT, D], bf16)

            for j in range(4):
                nc.sync.dma_start(hT[:, j, 0:T], hT_d[0, j])
            for j in range(4, 8):
                nc.scalar.dma_start(hT[:, j, 0:T], hT_d[0, j])
            nc.gpsimd.dma_start(
                wpk[:], wpack_d[:].rearrange("(j p) w -> p j w", p=128))
            nc.sync.dma_start(acol[:], acol_d[:])
            nc.sync.dma_start(bdt[:], bdt_d[:])
            nc.sync.dma_start(dsk[:], dsk_d[:])
            nc.sync.dma_start(nvth[:], nvth_d[:])
            nc.sync.dma_start(sel[:], sel_d[:])
            nc.scalar.dma_start(nhres[:],
                                nhres_d[:].rearrange("b t d -> t b d"))
            for j in range(0, 8, 2):
                nc.sync.dma_start(hT[:, j, T:R], hT_d[1, j])
                nc.scalar.dma_start(hT[:, j + 1, T:R], hT_d[1, j + 1])
            # out-stage weights: needed from ~mid-scan-phase onwards
            for j in range(0, 8, 2):
                nc.gpsimd.dma_start(
                    wout[:, j:j + 2, :],
                    wout_d[j * 128:(j + 2) * 128, :].rearrange(
                        "(j p) e -> p j e", p=128))

            # ---- persistent work tiles -----------------------------------
            xT = wpool.tile([128, R], bf16)
            dtT = wpool.tile([128, R], bf16)
            dtx = wpool.tile([128, R], bf16)
            gT = wpool.tile([128, R], bf16)
            bmcm = wpool.tile([32, R], bf16)
            gT_r = gT[:].rearrange("p (b t) -> p b t", b=B)

            out_stage_data = {}

            def emit_out_stage(b):
                ga, osb = out_stage_data[b]
                nh = nhres[:, b, :]
                for eh in range(2):
                    es = slice(eh * H, (eh + 1) * H)
                    po = pxpool.tile([128, H], f32, tag="pb", bufs=PB_BUFS)
                    for j in range(NCORES):
                        nc.tensor.matmul(po[:], ga[:, j, :], wout[:, j, es],
                                         start=(j == 0),
                                         stop=(j == NCORES - 1))
                    # fold the -h residual into the PSUM->SBUF staging copy
                    # (nhres is pre-negated bf16 on the host)
                    nc.vector.tensor_add(osb[:, es], po[:], nh[:, es])
                    nc.sync.dma_start(out_d[b][:, es], osb[:, es])

            def emit_proj(b, w0, w1, ptag, halves=(0, 1), pp=None):
                if pp is None:
                    pp = pxpool.tile([128 if ptag != "pm" else 32, T], f32,
                                     tag="pb", bufs=PB_BUFS)
                for hh in halves:
                    hs = slice(b * T + hh * H, b * T + (hh + 1) * H)
                    for j in range(KT):
                        nc.tensor.matmul(pp[:, hh * H:(hh + 1) * H],
                                         wpk[:, j, w0:w1], hT[:, j, hs],
                                         start=(j == 0), stop=(j == KT - 1))
                return pp

            def emit_proj_x(b):
                bs = slice(b * T, (b + 1) * T)
                px = emit_proj(b, 0, DL, "px")
                if b == 0:
                    # head-critical: DVE is idle here and shortens the chain
                    nc.vector.tensor_copy(xT[:, bs], px[:])
                else:
                    nc.scalar.activation(xT[:, bs], px[:], Act.Copy)

            def emit_proj_dt(b):
                bs = slice(b * T, (b + 1) * T)
                pd = emit_proj(b, 2 * DL, 3 * DL, "pd")
                # softplus(x+b) = ln(1 + exp(x+b)); Exp and Ln share a table
                et = scpool.tile([128, T], bf16, tag="et", bufs=2)
                nc.scalar.activation(et[:], pd[:], Act.Exp, bias=bdt[:, 0:1])
                nc.scalar.activation(dtT[:, bs], et[:], Act.Ln, bias=1.0)

            def emit_proj_bm(b):
                bs = slice(b * T, (b + 1) * T)
                pm = emit_proj(b, 3 * DL, WP, "pm")
                if b == 0:
                    nc.vector.tensor_copy(bmcm[:, bs], pm[:])
                else:
                    nc.scalar.activation(bmcm[:, bs], pm[:], Act.Copy)
                nc.vector.tensor_mul(dtx[:, bs], dtT[:, bs], xT[:, bs])

            # activation-table preheat: pulls the combined exp/ln table load
            # off the first softplus's critical path (Ln resolves uniquely to
            # natural_log_exp_and_others; Exp alone would pick exp_and_others)
            warm = scpool.tile([DL, 1], bf16, tag="warm", bufs=1)
            nc.scalar.activation(warm[:], nvth[:, 0:1], Act.Ln,
                                 scale=0.0, bias=1.0)

            emit_proj_x(0)
            emit_proj_dt(0)
            emit_proj_bm(0)
            proj1 = {}

            for b in range(B):
                bs = slice(b * T, (b + 1) * T)
                # b0's epilogue muls overlap b1's head on GpSimd; b1's run
                # on the then-idle DVE to keep the tail chain short
                epi = nc.gpsimd if b == 0 else nc.vector

                # ---- n-loop ---------------------------------------------
                y0 = scpool.tile([128, T], bf16, tag="y0", bufs=1)
                y1 = scpool.tile([128, T], bf16, tag="y1", bufs=1)
                zT_c = scpool.tile([128, T], bf16, tag="zc", bufs=1)

                for n in range(N):
                    decay = scpool.tile([128, T], bf16, tag="decay", bufs=6)
                    nc.scalar.activation(decay[:], dtT[:, bs], Act.Exp,
                                         scale=acol[:, n:n + 1])
                    pbm = pxpool.tile([128, T], f32, tag="pb", bufs=PB_BUFS)
                    for hh in range(2):
                        hs_d = slice(hh * H, (hh + 1) * H)
                        hs_s = slice(b * T + hh * H, b * T + (hh + 1) * H)
                        nc.tensor.matmul(pbm[:, hs_d],
                                         sel[:, n * 128:(n + 1) * 128],
                                         bmcm[:, hs_s], start=True, stop=True)
                    # inp = dtx*Bm in place in PSUM: the scan reads data1
                    # from PSUM at full speed (an SBUF data1 contends with
                    # GpSimd on the SBUF ports and runs ~1.8x slower)
                    nc.vector.tensor_mul(pbm[:], dtx[:, bs], pbm[:])
                    s = scpool.tile([128, T], bf16, tag="s", bufs=6)
                    nc.vector.tensor_tensor_scan(s[:], decay[:], pbm[:],
                                                 0.0, Alu.mult, Alu.add)
                    pcm = pxpool.tile([128, T], f32, tag="pb", bufs=PB_BUFS)
                    for hh in range(2):
                        hs_d = slice(hh * H, (hh + 1) * H)
                        hs_s = slice(b * T + hh * H, b * T + (hh + 1) * H)
                        nc.tensor.matmul(pcm[:, hs_d],
                                         sel[:, (N + n) * 128:(N + n + 1) * 128],
                                         bmcm[:, hs_s], start=True, stop=True)
                    cm = scpool.tile([128, T], bf16, tag="cm", bufs=6)
                    nc.scalar.activation(cm[:], pcm[:], Act.Copy)
                    # y accumulation: GpSimd owns two alternating chains;
                    # the last two n land on DVE to shorten the tail
                    if n == 0:
                        nc.gpsimd.tensor_mul(y0[:], s[:], cm[:])
                    elif n == 1:
                        nc.gpsimd.tensor_mul(y1[:], s[:], cm[:])
                    else:
                        eng = nc.vector if n >= N - DVE_TAIL[b] else nc.gpsimd
                        tmp = scpool.tile([128, T], bf16, tag="tmp", bufs=6)
                        eng.tensor_mul(tmp[:], s[:], cm[:])
                        acc = y1 if n % 2 == 1 else y0
                        eng.tensor_add(acc[:], acc[:], tmp[:])

                    # b1's projections, inserted one half-T chunk per n so
                    # PE's broadcast stream never starves DVE for long
                    if b == 0 and PROJ1_POS <= n < PROJ1_POS + 6:
                        step = n - PROJ1_POS
                        which, hh = divmod(step, 2)
                        w0, w1, ptag = [(0, DL, "px"), (2 * DL, 3 * DL, "pd"),
                                        (3 * DL, WP, "pm")][which]
                        proj1[ptag] = emit_proj(1, w0, w1, ptag, halves=(hh,),
                                                pp=proj1.get(ptag))
                        b1s = slice(T, R)
                        if step == 1:
                            nc.scalar.activation(xT[:, b1s], proj1["px"][:],
                                                 Act.Copy)
                        elif step == 3:
                            et1 = scpool.tile([128, T], bf16, tag="et",
                                              bufs=2)
                            nc.scalar.activation(et1[:], proj1["pd"][:],
                                                 Act.Exp, bias=bdt[:, 0:1])
                            nc.scalar.activation(dtT[:, b1s], et1[:],
                                                 Act.Ln, bias=1.0)
                        elif step == 5:
                            nc.scalar.activation(bmcm[:, b1s], proj1["pm"][:],
                                                 Act.Copy)
                            nc.vector.tensor_mul(dtx[:, b1s], dtT[:, b1s],
                                                 xT[:, b1s])
                    if n in (Z_POS, Z_POS + 2):
                        # z projection in two half-T chunks so PE's insert
                        # doesn't starve the broadcast stream
                        hh = 0 if n == Z_POS else 1
                        pz = pxpool.tile([128, H], f32, tag="pb", bufs=PB_BUFS)
                        hs = slice(b * T + hh * H, b * T + (hh + 1) * H)
                        for j in range(KT):
                            nc.tensor.matmul(pz[:], wpk[:, j, DL:2 * DL],
                                             hT[:, j, hs],
                                             start=(j == 0), stop=(j == KT - 1))
                        nc.scalar.activation(zT_c[:, hh * H:(hh + 1) * H],
                                             pz[:], Act.Copy)

                # ---- epilogue -------------------------------------------
                # y = y0+y1 ; y += D_skip*x ; spk = sigmoid(10y - 10vth)
                # g = y*spk*silu(z); both sigmoids adjacent (one table swap)
                nc.vector.tensor_add(y0[:], y0[:], y1[:])
                nc.vector.scalar_tensor_tensor(y0[:], xT[:, bs],
                                               dsk[:, 0:1], y0[:],
                                               Alu.mult, Alu.add)
                sgz = scpool.tile([128, T], bf16, tag="sgz", bufs=2)
                nc.scalar.activation(sgz[:], zT_c[:], Act.Sigmoid)
                spk = scpool.tile([128, T], bf16, tag="spk", bufs=2)
                nc.scalar.activation(spk[:], y0[:], Act.Sigmoid,
                                     scale=10.0, bias=nvth[:, 0:1])
                tz = scpool.tile([128, T], bf16, tag="tz", bufs=2)
                epi.tensor_mul(tz[:], sgz[:], zT_c[:])
                t1 = scpool.tile([128, T], bf16, tag="t1", bufs=2)
                epi.tensor_mul(t1[:], spk[:], tz[:])
                epi.tensor_mul(gT[:, bs], t1[:], y0[:])

                # ---- AllToAll this b's g: d-shards -> t-shards ----------
                a2a_in = dpool.tile([NCORES, DL, TL], bf16, tag=f"a2ai{b}")
                a2a_out = dpool.tile([NCORES, DL, TL], bf16, tag=f"a2ao{b}")
                nc.sync.dma_start(
                    a2a_in[:].rearrange("j p t -> p j t"),
                    gT_r[:, b, :].rearrange("p (j t) -> p j t", j=NCORES))
                nc.gpsimd.collective_compute(
                    "AllToAll",
                    Alu.bypass,
                    replica_groups=[list(range(NCORES))],
                    ins=[a2a_in[:].opt()],
                    outs=[a2a_out[:].opt()],
                )
                ga = wpool.tile([128, NCORES, TL], bf16, tag=f"ga{b}")
                nc.sync.dma_start(ga[:],
                                  a2a_out[:].rearrange("j p t -> p j t"))
                osb = wpool.tile([TL, D], f32, tag=f"osb{b}")
                out_stage_data[b] = (ga, osb)

            # out stages after all scan work: keeps PE busy through b1's
            # epilogue/AllToAll window (no pipeline drain before out1) and
            # keeps out0's staging off the mid-loop ACT/DVE streams
            emit_out_stage(0)
            emit_out_stage(1)

    nc.compile()
    _GRAPH_CACHE["nc"] = nc
    return nc


def _install_ntff_hook_shim():
    """This image's antenv package lacks axon_hooks; recreate it with the
    ctypes NTFF hook from trn_agent_boot so trace=True yields exec_time_ns."""
    import sys
    import types
    try:
        import antenv.axon_hooks  # noqa: F401
        return
    except ImportError:
        pass
    import antenv
    mod = types.ModuleType("antenv.axon_hooks")
    _h = {"v": None}
    mod.set_axon_ntff_profile_hook = lambda hook: _h.update(v=hook)
    mod.get_axon_ntff_profile_hook = lambda: _h["v"]
    sys.modules["antenv.axon_hooks"] = mod
    antenv.axon_hooks = mod
    try:
        from trn_agent_boot.trn_boot import _ntff_profile_via_ctypes
        hook = _ntff_profile_via_ctypes("/opt/axon/libaxon_pjrt.so")
        mod.set_axon_ntff_profile_hook(hook)
    except Exception as e:  # degrade to no-trace
        print(f"ntff hook shim failed: {e}")


def kernel(hidden_states, W_xz, W_dt, b_dt, A_log, W_B, W_C, D_skip, W_out,
           v_th):
    h = np.asarray(hidden_states, np.float32)
    Wxz = np.asarray(W_xz, np.float32)
    Wdt = np.asarray(W_dt, np.float32)
    bdt = np.asarray(b_dt, np.float32)
    Alog = np.asarray(A_log, np.float32)
    WB = np.asarray(W_B, np.float32)
    WC = np.asarray(W_C, np.float32)
    Dsk = np.asarray(D_skip, np.float32)
    Wout = np.asarray(W_out, np.float32)
    vth = np.asarray(v_th, np.float32)

    # [B, KT, 128, T] so each per-tile DMA reads one contiguous 256KB block
    hT = np.ascontiguousarray(
        h.transpose(2, 0, 1).reshape(KT, 128, B, T).transpose(2, 0, 1, 3)
    ).astype(BF16)
    Wxd = (Wxz[:, :D].astype(np.float64) @ Wdt.astype(np.float64)).astype(
        np.float32)
    A = -np.exp(Alog)
    wbc = np.concatenate([WB, WC], axis=1)
    wout_bf = Wout.astype(BF16)
    sel_np = np.zeros((2 * N, 2 * N * 128), dtype=BF16)
    for n in range(2 * N):
        sel_np[n, n * 128:(n + 1) * 128] = 1.0

    in_maps = []
    for k in range(NCORES):
        ds = slice(k * DL, (k + 1) * DL)
        ts = slice(k * TL, (k + 1) * TL)
        in_maps.append({
            "hT": hT,
            "wpack": np.ascontiguousarray(np.concatenate(
                [Wxz[:, :D][:, ds], Wxz[:, D:][:, ds], Wxd[:, ds], wbc],
                axis=1)).astype(BF16),
            "wout": wout_bf,
            "acol": np.ascontiguousarray(A[ds, :]),
            "bdt": np.ascontiguousarray(bdt[ds].reshape(DL, 1)),
            "dsk": np.ascontiguousarray(Dsk[ds].reshape(DL, 1)),
            "nvth": np.ascontiguousarray(
                (-10.0 * np.maximum(vth[ds], 0.1)).reshape(DL, 1)),
            "nhres": np.ascontiguousarray(-h[:, ts, :]).astype(BF16),
            "sel": sel_np,
        })

    from concourse.bass_utils import run_bass_kernel_spmd

    nc = _build_graph()
    trace = os.environ.get("KERNEL_TRACE", "0") == "1"
    kwargs = {}
    if trace:
        _install_ntff_hook_shim()
        import tempfile
        tmpdir = tempfile.mkdtemp(prefix="biossm_trace_")
        kwargs = dict(trace=True, tmpdir=tmpdir)
        LAST["trace_dir"] = tmpdir
    try:
        res = run_bass_kernel_spmd(nc, in_maps, core_ids=list(range(NCORES)),
                                   **kwargs)
    except Exception:
        # one retry: a crashed prior run can leave sticky device state that
        # clears on the next attempt
        res = run_bass_kernel_spmd(nc, in_maps, core_ids=list(range(NCORES)),
                                   **kwargs)
    LAST["exec_time_ns"] = getattr(res, "exec_time_ns", None)
    out = np.concatenate(
        [np.asarray(res.results[i]["out"], np.float32) for i in range(NCORES)],
        axis=1)
    return out


# revision 27
# speedup vs baseline: 1.0994x; 1.0994x over previous
"""BioSSMMixer distributed Trainium2 kernel (8 NeuronCores), v2.

Sharding: channel dimension D is split across the 8 cores (the SSM scan is
diagonal in D, so each core scans its own 128 channels with no cross-core
state). The final W_out projection contracts the full D, so the gate tensor
g = y_sp * silu(z) is exchanged with an AllToAll (d-shards -> t-shards) and
each core computes the output rows for its own T/8 slice.

v2 engine plan (v1 ran nearly everything heavy on DVE, which saturated at
~107% while GpSimd/ACT idled):
- DVE keeps the 16 recurrence scans per batch row (tensor_tensor_scan only
  lowers on DVE; 2 cycles/elem is its hard floor) plus the Bm input muls.
- GpSimd runs the s*Cm products and half the y accumulation chains.
- The Activation engine converts PSUM f32 broadcast results to bf16 SBUF
  (Copy lives in every act table, so no table swap next to the Exp decays);
  with bf16-only operands DVE tensor_tensor ops hit the 2x_1p perf mode.
- The -h residual is folded into the output matmul as one extra accumulating
  matmul (identity stationary, negated-h moving), removing the tail
  subtractions and the f32 hres load.
- b=0's output stage is emitted mid-b=1-n-loop so PE never stalls on the
  AllToAll; DMA issue is spread over the three DGE-capable queues with b=0's
  h tiles first.

Host-side prep (not part of HW exec time): W_xd = W_xz[:, :D] @ W_dt is
folded so dt comes directly from h; h is pre-transposed to [D, B*T] bf16.
"""

import os
import numpy as np
import ml_dtypes

B, T, D, N = 2, 1024, 1024, 16
NCORES = 8
DL = D // NCORES        # 128 channels per core
TL = T // NCORES        # 128 timesteps per core (output slice)
R = B * T               # 2048 rows, b-major: row = b*T + t
KT = D // 128           # 8 contraction tiles
H = 512                 # psum half-tile

BF16 = ml_dtypes.bfloat16

# Tuning knobs (engine placement).
Z_POS = 10              # first n at which a z-projection half is emitted
PROJ1_POS = 5           # n (in b=0 loop) where b=1's projections are emitted
# trailing n's whose s*Cm + accumulate run on DVE instead of GpSimd: GpSimd
# is ~3.3x slower per op than 2x-mode DVE, so its 2-ops-per-n chain falls
# behind the scan cadence and stalls everyone via the s/tmp pool slots
DVE_TAIL = (4, 5)
PB_BUFS = 3             # PSUM ring slots for projection/bcast tiles

# Filled by kernel() when KERNEL_TRACE=1: exec_time_ns, trace tmpdir.
LAST = {}

_GRAPH_CACHE = {}


def _patch_act_tables():
    """Order activation tables so Exp and Ln resolve to the combined
    natural_log_exp_and_others table (otherwise the table-load pass
    ping-pongs between exp_and_others and natural_log)."""
    import concourse.hw_specs as hw_specs
    import concourse.bacc as bacc_mod
    orig = hw_specs.get_activation_tables.__wrapped__
    import functools

    @functools.cache
    def reordered(arch):
        import concourse.mybir as mybir
        Act = mybir.ActivationFunctionType
        t = {k: set(v) for k, v in orig(arch).items()}
        if "natural_log_exp_and_others" in t:
            for k in ("exp_and_others", "exp_and_friends"):
                t.get(k, set()).discard(Act.Exp)
            t.get("natural_log", set()).discard(Act.Ln)
        return t

    hw_specs.get_activation_tables = reordered
    bacc_mod.get_activation_tables = reordered


def _build_graph():
    if "nc" in _GRAPH_CACHE:
        return _GRAPH_CACHE["nc"]

    import concourse.bacc as bacc
    import concourse.mybir as mybir
    from concourse import tile

    if os.environ.get('ACT_PATCH', '1') == '1':
        _patch_act_tables()

    f32 = mybir.dt.float32
    bf16 = mybir.dt.bfloat16
    Alu = mybir.AluOpType
    Act = mybir.ActivationFunctionType

    nc = bacc.Bacc(
        "TRN2",
        target_bir_lowering=False,
        debug=False,
        enable_asserts=True,
        num_devices=NCORES,
    )

    hT_d = nc.dram_tensor("hT", [B, KT, 128, T], bf16, kind="ExternalInput")
    wpack_d = nc.dram_tensor("wpack", [D, 3 * DL + 2 * N], bf16,
                             kind="ExternalInput")
    wout_d = nc.dram_tensor("wout", [D, D], bf16, kind="ExternalInput")
    acol_d = nc.dram_tensor("acol", [DL, N], f32, kind="ExternalInput")
    bdt_d = nc.dram_tensor("bdt", [DL, 1], f32, kind="ExternalInput")
    dsk_d = nc.dram_tensor("dsk", [DL, 1], f32, kind="ExternalInput")
    nvth_d = nc.dram_tensor("nvth", [DL, 1], f32, kind="ExternalInput")
    nhres_d = nc.dram_tensor("nhres", [B, TL, D], bf16, kind="ExternalInput")
    sel_d = nc.dram_tensor("sel", [2 * N, 2 * N * 128], bf16,
                           kind="ExternalInput")
    out_d = nc.dram_tensor("out", [B, TL, D], f32, kind="ExternalOutput")

    WP = 3 * DL + 2 * N

    with tile.TileContext(nc) as tc:
        with (
            tc.tile_pool(name="const", bufs=1) as cpool,
            tc.tile_pool(name="work", bufs=1) as wpool,
            tc.tile_pool(name="sc", bufs=4) as scpool,
            tc.tile_pool(name="px", bufs=2, space="PSUM") as pxpool,
            tc.tile_pool(name="dram", bufs=1, space="DRAM") as dpool,
        ):
            # ---- constant + input loads (b0's h first, spread queues) ----
            hT = cpool.tile([128, KT, R], bf16)
            wpk = cpool.tile([128, KT, WP], bf16)
            acol = cpool.tile([DL, N], f32)
            bdt = cpool.tile([DL, 1], f32)
            dsk = cpool.tile([DL, 1], f32)
            nvth = cpool.tile([DL, 1], f32)
            sel = cpool.tile([2 * N, 2 * N * 128], bf16)
            nhres = cpool.tile([TL, B, D], bf16)
            wout = cpool.tile([128, KT, D], bf16)

            for j in range(4):
                nc.sync.dma_start(hT[:, j, 0:T], hT_d[0, j])
            for j in range(4, 8):
                nc.scalar.dma_start(hT[:, j, 0:T], hT_d[0, j])
            nc.gpsimd.dma_start(
                wpk[:], wpack_d[:].rearrange("(j p) w -> p j w", p=128))
            nc.sync.dma_start(acol[:], acol_d[:])
            nc.sync.dma_start(bdt[:], bdt_d[:])
            nc.sync.dma_start(dsk[:], dsk_d[:])
            nc.sync.dma_start(nvth[:], nvth_d[:])
            nc.sync.dma_start(sel[:], sel_d[:])
            nc.scalar.dma_start(nhres[:],
                                nhres_d[:].rearrange("b t d -> t b d"))
            for j in range(0, 8, 2):
                nc.sync.dma_start(hT[:, j, T:R], hT_d[1, j])
                nc.scalar.dma_start(hT[:, j + 1, T:R], hT_d[1, j + 1])
            # out-stage weights: needed from ~mid-scan-phase onwards
            for j in range(0, 8, 2):
                nc.gpsimd.dma_start(
                    wout[:, j:j + 2, :],
                    wout_d[j * 128:(j + 2) * 128, :].rearrange(
                        "(j p) e -> p j e", p=128))

            # ---- persistent work tiles -----------------------------------
            xT = wpool.tile([128, R], bf16)
            dtT = wpool.tile([128, R], bf16)
            dtx = wpool.tile([128, R], bf16)
            gT = wpool.tile([128, R], bf16)
            bmcm = wpool.tile([32, R], bf16)
            gT_r = gT[:].rearrange("p (b t) -> p b t", b=B)

            out_stage_data = {}

            def emit_out_stage(b):
                ga, osb = out_stage_data[b]
                nh = nhres[:, b, :]
                for eh in range(2):
                    es = slice(eh * H, (eh + 1) * H)
                    po = pxpool.tile([128, H], f32, tag="po")
                    for j in range(NCORES):
                        nc.tensor.matmul(po[:], ga[:, j, :], wout[:, j, es],
                                         start=(j == 0),
                                         stop=(j == NCORES - 1))
                    # fold the -h residual into the PSUM->SBUF staging copy
                    # (nhres is pre-negated bf16 on the host)
                    nc.vector.tensor_add(osb[:, es], po[:], nh[:, es])
                    nc.sync.dma_start(out_d[b][:, es], osb[:, es])

            def emit_proj(b, w0, w1, ptag, halves=(0, 1), pp=None):
                if pp is None:
                    pp = pxpool.tile([128 if ptag != "pm" else 32, T], f32,
                                     tag="pb", bufs=PB_BUFS)
                for hh in halves:
                    hs = slice(b * T + hh * H, b * T + (hh + 1) * H)
                    for j in range(KT):
                        nc.tensor.matmul(pp[:, hh * H:(hh + 1) * H],
                                         wpk[:, j, w0:w1], hT[:, j, hs],
                                         start=(j == 0), stop=(j == KT - 1))
                return pp

            def emit_proj_x(b):
                bs = slice(b * T, (b + 1) * T)
                px = emit_proj(b, 0, DL, "px")
                if b == 0:
                    # head-critical: DVE is idle here and shortens the chain
                    nc.vector.tensor_copy(xT[:, bs], px[:])
                else:
                    nc.scalar.activation(xT[:, bs], px[:], Act.Copy)

            def emit_proj_dt(b):
                bs = slice(b * T, (b + 1) * T)
                pd = emit_proj(b, 2 * DL, 3 * DL, "pd")
                # softplus(x+b) = ln(1 + exp(x+b)); Exp and Ln share a table
                et = scpool.tile([128, T], bf16, tag="et", bufs=2)
                nc.scalar.activation(et[:], pd[:], Act.Exp, bias=bdt[:, 0:1])
                nc.scalar.activation(dtT[:, bs], et[:], Act.Ln, bias=1.0)

            def emit_proj_bm(b):
                bs = slice(b * T, (b + 1) * T)
                pm = emit_proj(b, 3 * DL, WP, "pm")
                if b == 0:
                    nc.vector.tensor_copy(bmcm[:, bs], pm[:])
                else:
                    nc.scalar.activation(bmcm[:, bs], pm[:], Act.Copy)
                nc.vector.tensor_mul(dtx[:, bs], dtT[:, bs], xT[:, bs])

            # activation-table preheat: pulls the combined exp/ln table load
            # off the first softplus's critical path (Ln resolves uniquely to
            # natural_log_exp_and_others; Exp alone would pick exp_and_others)
            warm = scpool.tile([DL, 1], bf16, tag="warm", bufs=1)
            nc.scalar.activation(warm[:], nvth[:, 0:1], Act.Ln,
                                 scale=0.0, bias=1.0)

            emit_proj_x(0)
            emit_proj_dt(0)
            emit_proj_bm(0)
            proj1 = {}

            for b in range(B):
                bs = slice(b * T, (b + 1) * T)
                # b0's epilogue muls overlap b1's head on GpSimd; b1's run
                # on the then-idle DVE to keep the tail chain short
                epi = nc.gpsimd if b == 0 else nc.vector

                # ---- n-loop ---------------------------------------------
                y0 = scpool.tile([128, T], bf16, tag="y0", bufs=1)
                y1 = scpool.tile([128, T], bf16, tag="y1", bufs=1)
                zT_c = scpool.tile([128, T], bf16, tag="zc", bufs=1)

                for n in range(N):
                    decay = scpool.tile([128, T], bf16, tag="decay", bufs=6)
                    nc.scalar.activation(decay[:], dtT[:, bs], Act.Exp,
                                         scale=acol[:, n:n + 1])
                    pbm = pxpool.tile([128, T], f32, tag="pb", bufs=PB_BUFS)
                    for hh in range(2):
                        hs_d = slice(hh * H, (hh + 1) * H)
                        hs_s = slice(b * T + hh * H, b * T + (hh + 1) * H)
                        nc.tensor.matmul(pbm[:, hs_d],
                                         sel[:, n * 128:(n + 1) * 128],
                                         bmcm[:, hs_s], start=True, stop=True)
                    # inp = dtx*Bm in place in PSUM: the scan reads data1
                    # from PSUM at full speed (an SBUF data1 contends with
                    # GpSimd on the SBUF ports and runs ~1.8x slower)
                    nc.vector.tensor_mul(pbm[:], dtx[:, bs], pbm[:])
                    s = scpool.tile([128, T], bf16, tag="s", bufs=6)
                    nc.vector.tensor_tensor_scan(s[:], decay[:], pbm[:],
                                                 0.0, Alu.mult, Alu.add)
                    pcm = pxpool.tile([128, T], f32, tag="pb", bufs=PB_BUFS)
                    for hh in range(2):
                        hs_d = slice(hh * H, (hh + 1) * H)
                        hs_s = slice(b * T + hh * H, b * T + (hh + 1) * H)
                        nc.tensor.matmul(pcm[:, hs_d],
                                         sel[:, (N + n) * 128:(N + n + 1) * 128],
                                         bmcm[:, hs_s], start=True, stop=True)
                    cm = scpool.tile([128, T], bf16, tag="cm", bufs=6)
                    nc.scalar.activation(cm[:], pcm[:], Act.Copy)
                    # y accumulation: GpSimd owns two alternating chains;
                    # the last two n land on DVE to shorten the tail
                    if n == 0:
                        nc.gpsimd.tensor_mul(y0[:], s[:], cm[:])
                    elif n == 1:
                        nc.gpsimd.tensor_mul(y1[:], s[:], cm[:])
                    else:
                        eng = nc.vector if n >= N - DVE_TAIL[b] else nc.gpsimd
                        tmp = scpool.tile([128, T], bf16, tag="tmp", bufs=6)
                        eng.tensor_mul(tmp[:], s[:], cm[:])
                        acc = y1 if n % 2 == 1 else y0
                        eng.tensor_add(acc[:], acc[:], tmp[:])

                    if b == 0 and n == PROJ1_POS:
                        emit_proj_x(1)
                    if b == 0 and n == PROJ1_POS + 2:
                        emit_proj_dt(1)
                    if b == 0 and n == PROJ1_POS + 4:
                        emit_proj_bm(1)
                    if n in (Z_POS, Z_POS + 2):
                        # z projection in two half-T chunks so PE's insert
                        # doesn't starve the broadcast stream
                        hh = 0 if n == Z_POS else 1
                        pz = pxpool.tile([128, H], f32, tag="pb", bufs=PB_BUFS)
                        hs = slice(b * T + hh * H, b * T + (hh + 1) * H)
                        for j in range(KT):
                            nc.tensor.matmul(pz[:], wpk[:, j, DL:2 * DL],
                                             hT[:, j, hs],
                                             start=(j == 0), stop=(j == KT - 1))
                        nc.scalar.activation(zT_c[:, hh * H:(hh + 1) * H],
                                             pz[:], Act.Copy)

                # ---- epilogue -------------------------------------------
                # y = y0+y1 ; y += D_skip*x ; spk = sigmoid(10y - 10vth)
                # g = y*spk*silu(z); both sigmoids adjacent (one table swap)
                nc.vector.tensor_add(y0[:], y0[:], y1[:])
                nc.vector.scalar_tensor_tensor(y0[:], xT[:, bs],
                                               dsk[:, 0:1], y0[:],
                                               Alu.mult, Alu.add)
                sgz = scpool.tile([128, T], bf16, tag="sgz", bufs=2)
                nc.scalar.activation(sgz[:], zT_c[:], Act.Sigmoid)
                spk = scpool.tile([128, T], bf16, tag="spk", bufs=2)
                nc.scalar.activation(spk[:], y0[:], Act.Sigmoid,
                                     scale=10.0, bias=nvth[:, 0:1])
                tz = scpool.tile([128, T], bf16, tag="tz", bufs=2)
                epi.tensor_mul(tz[:], sgz[:], zT_c[:])
                t1 = scpool.tile([128, T], bf16, tag="t1", bufs=2)
                epi.tensor_mul(t1[:], spk[:], tz[:])
                epi.tensor_mul(gT[:, bs], t1[:], y0[:])

                # ---- AllToAll this b's g: d-shards -> t-shards ----------
                a2a_in = dpool.tile([NCORES, DL, TL], bf16, tag=f"a2ai{b}")
                a2a_out = dpool.tile([NCORES, DL, TL], bf16, tag=f"a2ao{b}")
                nc.sync.dma_start(
                    a2a_in[:].rearrange("j p t -> p j t"),
                    gT_r[:, b, :].rearrange("p (j t) -> p j t", j=NCORES))
                nc.gpsimd.collective_compute(
                    "AllToAll",
                    Alu.bypass,
                    replica_groups=[list(range(NCORES))],
                    ins=[a2a_in[:].opt()],
                    outs=[a2a_out[:].opt()],
                )
                ga = wpool.tile([128, NCORES, TL], bf16, tag=f"ga{b}")
                nc.sync.dma_start(ga[:],
                                  a2a_out[:].rearrange("j p t -> p j t"))
                osb = wpool.tile([TL, D], f32, tag=f"osb{b}")
                out_stage_data[b] = (ga, osb)

            # out stages after all scan work: keeps PE busy through b1's
            # epilogue/AllToAll window (no pipeline drain before out1) and
            # keeps out0's staging off the mid-loop ACT/DVE streams
            emit_out_stage(0)
            emit_out_stage(1)

    nc.compile()
    _GRAPH_CACHE["nc"] = nc
    return nc


def _install_ntff_hook_shim():
    """This image's antenv package lacks axon_hooks; recreate it with the
    ctypes NTFF hook from trn_agent_boot so trace=True yields exec_time_ns."""
    import sys
    import types
    try:
        import antenv.axon_hooks  # noqa: F401
        return
    except ImportError:
        pass
    import antenv
    mod = types.ModuleType("antenv.axon_hooks")
    _h = {"v": None}
    mod.set_axon_ntff_profile_hook = lambda hook: _h.update(v=hook)
    mod.get_axon_ntff_profile_hook = lambda: _h["v"]
    sys.modules["antenv.axon_hooks"] = mod
    antenv.axon_hooks = mod
    try:
        from trn_agent_boot.trn_boot import _ntff_profile_via_ctypes
        hook = _ntff_profile_via_ctypes("/opt/axon/libaxon_pjrt.so")
        mod.set_axon_ntff_profile_hook(hook)
    except Exception as e:  # degrade to no-trace
        print(f"ntff hook shim failed: {e}")


def kernel(hidden_states, W_xz, W_dt, b_dt, A_log, W_B, W_C, D_skip, W_out,
           v_th):
    h = np.asarray(hidden_states, np.float32)
    Wxz = np.asarray(W_xz, np.float32)
    Wdt = np.asarray(W_dt, np.float32)
    bdt = np.asarray(b_dt, np.float32)
    Alog = np.asarray(A_log, np.float32)
    WB = np.asarray(W_B, np.float32)
    WC = np.asarray(W_C, np.float32)
    Dsk = np.asarray(D_skip, np.float32)
    Wout = np.asarray(W_out, np.float32)
    vth = np.asarray(v_th, np.float32)

    # [B, KT, 128, T] so each per-tile DMA reads one contiguous 256KB block
    hT = np.ascontiguousarray(
        h.transpose(2, 0, 1).reshape(KT, 128, B, T).transpose(2, 0, 1, 3)
    ).astype(BF16)
    Wxd = (Wxz[:, :D].astype(np.float64) @ Wdt.astype(np.float64)).astype(
        np.float32)
    A = -np.exp(Alog)
    wbc = np.concatenate([WB, WC], axis=1)
    wout_bf = Wout.astype(BF16)
    sel_np = np.zeros((2 * N, 2 * N * 128), dtype=BF16)
    for n in range(2 * N):
        sel_np[n, n * 128:(n + 1) * 128] = 1.0

    in_maps = []
    for k in range(NCORES):
        ds = slice(k * DL, (k + 1) * DL)
        ts = slice(k * TL, (k + 1) * TL)
        in_maps.append({
            "hT": hT,
            "wpack": np.ascontiguousarray(np.concatenate(
                [Wxz[:, :D][:, ds], Wxz[:, D:][:, ds], Wxd[:, ds], wbc],
                axis=1)).astype(BF16),
            "wout": wout_bf,
            "acol": np.ascontiguousarray(A[ds, :]),
            "bdt": np.ascontiguousarray(bdt[ds].reshape(DL, 1)),
            "dsk": np.ascontiguousarray(Dsk[ds].reshape(DL, 1)),
            "nvth": np.ascontiguousarray(
                (-10.0 * np.maximum(vth[ds], 0.1)).reshape(DL, 1)),
            "nhres": np.ascontiguousarray(-h[:, ts, :]).astype(BF16),
            "sel": sel_np,
        })

    from concourse.bass_utils import run_bass_kernel_spmd

    nc = _build_graph()
    trace = os.environ.get("KERNEL_TRACE", "0") == "1"
    kwargs = {}
    if trace:
        _install_ntff_hook_shim()
        import tempfile
        tmpdir = tempfile.mkdtemp(prefix="biossm_trace_")
        kwargs = dict(trace=True, tmpdir=tmpdir)
        LAST["trace_dir"] = tmpdir
    try:
        res = run_bass_kernel_spmd(nc, in_maps, core_ids=list(range(NCORES)),
                                   **kwargs)
    except Exception:
        # one retry: a crashed prior run can leave sticky device state that
        # clears on the next attempt
        res = run_bass_kernel_spmd(nc, in_maps, core_ids=list(range(NCORES)),
                                   **kwargs)
    LAST["exec_time_ns"] = getattr(res, "exec_time_ns", None)
    out = np.concatenate(
        [np.asarray(res.results[i]["out"], np.float32) for i in range(NCORES)],
        axis=1)
    return out
